# revision 1
# baseline (speedup 1.0000x reference)
"""Grouped MLP (MoE expert FFN) Bass kernel for 8 Trainium2 NeuronCores.

Problem: 4096 tokens sorted by expert (8 experts, uneven counts), per-expert
GLU MLP:  h = x @ w1[g]  (-> up|gate, 2*2048 cols);  a = silu(up)*gate;
y = a @ w2[g].

Sharding: expert-parallel.  Core g handles expert g's tokens (padded to a
common Tpad so all 8 cores run one identical program).  All shard/gather work
happens on the host; there are no device collectives.

Device program (per core), everything in transposed "feature-major" space so
weights are the stationary matmul operand with natural layouts:
  h^T[mi]  = sum_kc w1[kc, mi]^T @ x^T[kc]      (PSUM accum over K=1024)
  hgl[mi]  = silu(up) * gate                    (ACT + DVE, PSUM->SBUF)
  y^T[mo]  = sum_ki w2[ki, mo]^T @ hgl[ki]      (PSUM accum over K=2048)
Matmuls run as float32r (full PE rate at free-dim >= 256) on fp32 data.
"""

import sys

try:  # concourse normally comes from the container's PYTHONPATH
    import concourse  # noqa: F401
except ImportError:  # pragma: no cover - fallback for stripped env
    for _p in (
        "/root/.axon_site",
        "/root/.axon_site/_ro/trn_rl_repo",
        "/root/.axon_site/_ro/pypackages",
        "/opt/trn_rl_repo",
    ):
        if _p not in sys.path:
            sys.path.append(_p)

from contextlib import ExitStack

import numpy as np

NUM_TOKENS = 4096
HIDDEN = 1024
INTER = 2048
GROUPS = 8
N_CORES = 8

F32 = None  # set lazily after imports


def _ceil_to(x: int, m: int) -> int:
    return ((x + m - 1) // m) * m


_PROGRAM_CACHE: dict = {}


def _build_program(tpad: int):
    """Build + compile the single-core Bass program (same NEFF on all cores)."""
    import concourse.bass as bass  # noqa: F401
    import concourse.mybir as mybir
    import concourse.tile as tile
    from concourse import bacc

    f32 = mybir.dt.float32
    f32r = mybir.dt.float32r
    silu = mybir.ActivationFunctionType.Silu

    KC = HIDDEN // 128          # 8  k-blocks for fc1
    MI = INTER // 128           # 16 output row-blocks of h (pairs up+gate)
    KI = INTER // 128           # 16 k-blocks for fc2
    MO = HIDDEN // 128          # 8  output row-blocks of y

    # token chunks (free dim per matmul; <=512 for 4-byte dtypes)
    nts = []
    off = 0
    while off < tpad:
        nl = min(512, tpad - off)
        nts.append((off, nl))
        off += nl

    nc = bacc.Bacc("TRN2", target_bir_lowering=False, debug=False)

    xT_d = nc.dram_tensor("xT", [KC, 128, tpad], f32r, kind="ExternalInput").ap()
    w1_d = nc.dram_tensor("w1c", [MI, KC, 128, 256], f32r, kind="ExternalInput").ap()
    w2_d = nc.dram_tensor("w2c", [MO, KI, 128, 128], f32r, kind="ExternalInput").ap()
    y_d = nc.dram_tensor("yT", [MO, 128, tpad], f32, kind="ExternalOutput").ap()

    with tile.TileContext(nc) as tc, ExitStack() as ctx:
        xp = ctx.enter_context(tc.tile_pool(name="x", bufs=1))
        hp = ctx.enter_context(tc.tile_pool(name="hgl", bufs=1))
        yp = ctx.enter_context(tc.tile_pool(name="y", bufs=1))
        w1p = ctx.enter_context(tc.tile_pool(name="w1", bufs=3))
        w2p = ctx.enter_context(tc.tile_pool(name="w2", bufs=3))
        pup = ctx.enter_context(tc.tile_pool(name="pu", bufs=2, space="PSUM"))
        pgp = ctx.enter_context(tc.tile_pool(name="pg", bufs=2, space="PSUM"))
        pyp = ctx.enter_context(tc.tile_pool(name="py", bufs=2, space="PSUM"))
        tp = ctx.enter_context(tc.tile_pool(name="tmp", bufs=3))

        x_sb = xp.tile([128, KC * tpad], f32r)
        for kc in range(KC):
            nc.sync.dma_start(
                out=x_sb[:, kc * tpad : (kc + 1) * tpad], in_=xT_d[kc]
            )

        hgl = hp.tile([128, KI * tpad], f32r)
        y_sb = yp.tile([128, MO * tpad], f32)

        # ---- fc1 + GLU ----
        for mi in range(MI):
            w1t = w1p.tile([128, KC * 256], f32r)
            for kc in range(KC):
                nc.sync.dma_start(
                    out=w1t[:, kc * 256 : (kc + 1) * 256], in_=w1_d[mi, kc]
                )
            for no, nl in nts:
                pu = pup.tile([128, 512], f32)
                pg = pgp.tile([128, 512], f32)
                for kc in range(KC):
                    rhs = x_sb[:, kc * tpad + no : kc * tpad + no + nl]
                    nc.tensor.matmul(
                        pu[:, :nl],
                        w1t[:, kc * 256 : kc * 256 + 128],
                        rhs,
                        start=(kc == 0),
                        stop=(kc == KC - 1),
                    )
                    nc.tensor.matmul(
                        pg[:, :nl],
                        w1t[:, kc * 256 + 128 : kc * 256 + 256],
                        rhs,
                        start=(kc == 0),
                        stop=(kc == KC - 1),
                    )
                tmp = tp.tile([128, 512], f32)
                nc.scalar.activation(tmp[:, :nl], pu[:, :nl], silu)
                nc.vector.tensor_mul(
                    hgl[:, mi * tpad + no : mi * tpad + no + nl],
                    tmp[:, :nl],
                    pg[:, :nl],
                )

        # ---- fc2 ----
        for mo in range(MO):
            w2t = w2p.tile([128, KI * 128], f32r)
            for ki in range(KI):
                nc.sync.dma_start(
                    out=w2t[:, ki * 128 : (ki + 1) * 128], in_=w2_d[mo, ki]
                )
            for no, nl in nts:
                py = pyp.tile([128, 512], f32)
                for ki in range(KI):
                    nc.tensor.matmul(
                        py[:, :nl],
                        w2t[:, ki * 128 : ki * 128 + 128],
                        hgl[:, ki * tpad + no : ki * tpad + no + nl],
                        start=(ki == 0),
                        stop=(ki == KI - 1),
                    )
                nc.scalar.copy(
                    y_sb[:, mo * tpad + no : mo * tpad + no + nl], py[:, :nl]
                )

        for mo in range(MO):
            nc.sync.dma_start(
                out=y_d[mo], in_=y_sb[:, mo * tpad : (mo + 1) * tpad]
            )

    nc.compile()
    return nc


def _get_program(tpad: int):
    if tpad not in _PROGRAM_CACHE:
        _PROGRAM_CACHE[tpad] = _build_program(tpad)
    return _PROGRAM_CACHE[tpad]


def _prep_core_inputs(x_seg: np.ndarray, w1_g: np.ndarray, w2_g: np.ndarray, tpad: int):
    """Host-side shard prep for one core: transpose/pad tokens, retile weights."""
    cnt = x_seg.shape[0]
    xT = np.zeros((HIDDEN, tpad), np.float32)
    if cnt:
        xT[:, :cnt] = x_seg.T
    xT = np.ascontiguousarray(xT.reshape(HIDDEN // 128, 128, tpad))

    # w1_g: [1024, 4096] cols = up[0:2048] | gate[2048:4096]
    # -> [mi 16, kc 8, 128, 256] where cols 0:128 = up(mi), 128:256 = gate(mi)
    w1c = np.ascontiguousarray(
        w1_g.reshape(8, 128, 2, 16, 128).transpose(3, 0, 1, 2, 4).reshape(16, 8, 128, 256)
    )
    # w2_g: [2048, 1024] -> [mo 8, ki 16, 128, 128]
    w2c = np.ascontiguousarray(
        w2_g.reshape(16, 128, 8, 128).transpose(2, 0, 1, 3)
    )
    return {"xT": xT, "w1c": w1c, "w2c": w2c}


_LAST_RESULTS = {}  # exposed for test.py (exec time, trace paths)


def kernel(permuted_tokens, tokens_per_expert, w1, w2, _trace=False):
    from concourse.bass_utils import run_bass_kernel_spmd

    x = np.asarray(permuted_tokens, np.float32)
    counts = np.asarray(tokens_per_expert, np.int64)
    w1 = np.asarray(w1, np.float32)
    w2 = np.asarray(w2, np.float32)

    offs = np.zeros(GROUPS + 1, np.int64)
    offs[1:] = np.cumsum(counts)
    tpad = max(256, _ceil_to(int(counts.max()), 128))

    nc = _get_program(tpad)

    in_maps = []
    for g in range(GROUPS):
        in_maps.append(
            _prep_core_inputs(x[offs[g] : offs[g + 1]], w1[g], w2[g], tpad)
        )

    kwargs = {}
    if _trace:
        kwargs = dict(trace=True, trace_cores=list(range(N_CORES)))
    res = run_bass_kernel_spmd(nc, in_maps, core_ids=list(range(N_CORES)), **kwargs)
    _LAST_RESULTS["res"] = res

    out = np.empty((x.shape[0], HIDDEN), np.float32)
    for g in range(GROUPS):
        cnt = int(counts[g])
        if cnt == 0:
            continue
        yT = res.results[g]["yT"].reshape(HIDDEN, tpad)
        out[offs[g] : offs[g + 1]] = yT[:, :cnt].T
    return out



# revision 5
# speedup vs baseline: 1.8005x; 1.8005x over previous
"""Grouped MLP (MoE expert FFN) Bass kernel for 8 Trainium2 NeuronCores.

Problem: 4096 tokens sorted by expert (8 experts, uneven counts), per-expert
GLU MLP:  h = x @ w1[g]  (-> up|gate, 2*2048 cols);  a = silu(up)*gate;
y = a @ w2[g].

Sharding: 2-way token-parallel x 4-way tensor-parallel (INTER split), bf16.
Core (t, q) with t = c//4, q = c%4:
  - token group t owns 4 experts (balanced 4+4 partition of the counts),
  - inter slice q owns up/gate columns [q*512:(q+1)*512] of every expert
    and the matching w2 rows; its fc2 output is a partial sum of y.
Each core therefore runs an identical program over `slot` token segments
(common padded lengths across both groups); only the DRAM bytes differ.
The host converts everything to bf16, packs weight tiles in consumption
order, and sums the 4 partial y outputs per token group (host-side
reduction; no device collectives).

Device program per core (transposed feature-major space):
  fc1: for slot s, chunk c (<=512 tokens), pair p (128 inter cols):
       up/gate psum accumulated over 8 k-blocks; silu(up)*gate -> a (bf16)
  fc2: one slot behind fc1 (PE never waits on ACT/DVE):
       y^T psum over 4 k-blocks of a; copied bf16 to staging, DMA'd out.
"""

import sys

try:  # concourse normally comes from the container's PYTHONPATH
    import concourse  # noqa: F401
except ImportError:  # pragma: no cover - fallback for stripped env
    for _p in (
        "/root/.axon_site",
        "/root/.axon_site/_ro/trn_rl_repo",
        "/root/.axon_site/_ro/pypackages",
        "/opt/trn_rl_repo",
    ):
        if _p not in sys.path:
            sys.path.append(_p)

from contextlib import ExitStack

import numpy as np
import ml_dtypes

BF16NP = ml_dtypes.bfloat16

NUM_TOKENS = 4096
HIDDEN = 1024
INTER = 2048
GROUPS = 8
N_CORES = 8

NQ = 4              # inter-dim splits
NT = 2              # token-group splits
IW = INTER // NQ    # 512 inter cols per core
NPAIR = IW // 128   # 4 (up,gate) pair blocks
KC = HIDDEN // 128  # 8 k-blocks for fc1
KI = IW // 128      # 4 k-blocks for fc2
MO = HIDDEN // 128  # 8 output row-blocks of y
NSLOT = GROUPS // NT  # 4 expert slots per core


def _chunks(length):
    out, off = [], 0
    while off < length:
        c = min(512, length - off)
        out.append((off, c))
        off += c
    return out


_PROGRAM_CACHE: dict = {}


def _build_program(slot_lens):
    """Single-core Bass program; identical NEFF on all 8 cores."""
    import concourse.bass as bass  # noqa: F401
    import concourse.mybir as mybir
    import concourse.tile as tile
    from concourse import bacc

    f32 = mybir.dt.float32
    bf16 = mybir.dt.bfloat16
    silu = mybir.ActivationFunctionType.Silu

    tloc = sum(slot_lens)
    soffs = np.concatenate([[0], np.cumsum(slot_lens)]).astype(int)

    nc = bacc.Bacc("TRN2", target_bir_lowering=False, debug=False)

    xt_d = nc.dram_tensor("xt", [KC, 128, tloc], bf16, kind="ExternalInput").ap()
    w1_d = nc.dram_tensor(
        "w1b", [NSLOT, 128, NPAIR * 2 * KC * 128], bf16, kind="ExternalInput"
    ).ap()
    w2_d = nc.dram_tensor(
        "w2b", [NSLOT, 128, MO * KI * 128], bf16, kind="ExternalInput"
    ).ap()
    yt_d = nc.dram_tensor("yt", [128, MO, tloc], bf16, kind="ExternalOutput").ap()

    with tile.TileContext(nc) as tc, ExitStack() as ctx:
        xp = ctx.enter_context(tc.tile_pool(name="x", bufs=1))
        w1p = ctx.enter_context(tc.tile_pool(name="w1", bufs=2))
        w2p = ctx.enter_context(tc.tile_pool(name="w2", bufs=2))
        ap_ = ctx.enter_context(tc.tile_pool(name="a", bufs=2))
        yp = ctx.enter_context(tc.tile_pool(name="y", bufs=2))
        tp = ctx.enter_context(tc.tile_pool(name="tmp", bufs=4))
        pup = ctx.enter_context(tc.tile_pool(name="pu", bufs=2, space="PSUM"))
        pgp = ctx.enter_context(tc.tile_pool(name="pg", bufs=2, space="PSUM"))
        pyp = ctx.enter_context(tc.tile_pool(name="py", bufs=3, space="PSUM"))

        x_sb = xp.tile([128, KC, tloc], bf16)

        def emit_fc2(s, llen, w2t, a_t, copy_flip):
            y_t = yp.tile([128, MO, llen], bf16)
            for coff, clen in _chunks(llen):
                for mo in range(MO):
                    py = pyp.tile([128, clen], f32)
                    for ki in range(KI):
                        nc.tensor.matmul(
                            py[:, :],
                            w2t[:, (mo * KI + ki) * 128 : (mo * KI + ki + 1) * 128],
                            a_t[:, ki, coff : coff + clen],
                            start=(ki == 0),
                            stop=(ki == KI - 1),
                        )
                    dst = y_t[:, mo, coff : coff + clen]
                    if copy_flip[0]:
                        nc.scalar.copy(dst, py[:, :])
                    else:
                        nc.vector.tensor_copy(dst, py[:, :])
                    copy_flip[0] = not copy_flip[0]
            nc.sync.dma_start(
                out=yt_d[:, :, soffs[s] : soffs[s] + llen], in_=y_t[:, :, :]
            )

        pending = None
        copy_flip = [True]
        for s in range(NSLOT):
            llen = int(slot_lens[s])
            if llen == 0:
                continue
            w1t = w1p.tile([128, NPAIR * 2 * KC * 128], bf16)
            # first pair's weights + this slot's x land first so PE starts early
            nc.sync.dma_start(
                out=w1t[:, 0 : 2 * KC * 128], in_=w1_d[s, :, 0 : 2 * KC * 128]
            )
            for kc in range(KC):
                nc.sync.dma_start(
                    out=x_sb[:, kc, soffs[s] : soffs[s] + llen],
                    in_=xt_d[kc, :, soffs[s] : soffs[s] + llen],
                )
            for p in range(1, NPAIR):
                nc.sync.dma_start(
                    out=w1t[:, p * 2 * KC * 128 : (p + 1) * 2 * KC * 128],
                    in_=w1_d[s, :, p * 2 * KC * 128 : (p + 1) * 2 * KC * 128],
                )
            w2t = w2p.tile([128, MO * KI * 128], bf16)
            nc.sync.dma_start(out=w2t[:, :], in_=w2_d[s])

            a_t = ap_.tile([128, KI, llen], bf16)
            for coff, clen in _chunks(llen):
                for p in range(NPAIR):
                    pu = pup.tile([128, clen], f32)
                    pg = pgp.tile([128, clen], f32)
                    for h, ps in ((0, pu), (1, pg)):
                        tbase = (p * 2 + h) * KC
                        for kc in range(KC):
                            nc.tensor.matmul(
                                ps[:, :],
                                w1t[:, (tbase + kc) * 128 : (tbase + kc + 1) * 128],
                                x_sb[:, kc, soffs[s] + coff : soffs[s] + coff + clen],
                                start=(kc == 0),
                                stop=(kc == KC - 1),
                            )
                    tmp = tp.tile([128, clen], f32)
                    nc.scalar.activation(tmp[:, :], pu[:, :], silu)
                    nc.vector.tensor_mul(a_t[:, p, coff : coff + clen], tmp[:, :], pg[:, :])

            if pending is not None:
                emit_fc2(*pending, copy_flip)
            pending = (s, llen, w2t, a_t)
        if pending is not None:
            emit_fc2(*pending, copy_flip)

    nc.compile()
    return nc


def _get_program(slot_lens):
    key = tuple(int(v) for v in slot_lens)
    if key not in _PROGRAM_CACHE:
        _PROGRAM_CACHE[key] = _build_program(key)
    return _PROGRAM_CACHE[key]


def _partition_experts(counts):
    """Split experts into NT groups of GROUPS//NT, balancing token sums."""
    order = np.argsort(-counts, kind="stable")
    groups = [[] for _ in range(NT)]
    sums = [0] * NT
    cap = GROUPS // NT
    for e in order:
        cand = sorted(range(NT), key=lambda t: (sums[t],))
        for t in cand:
            if len(groups[t]) < cap:
                groups[t].append(int(e))
                sums[t] += int(counts[e])
                break
    # within each group, biggest expert first -> slot i
    for t in range(NT):
        groups[t].sort(key=lambda e: -int(counts[e]))
    slot_lens = tuple(
        max(int(counts[groups[t][i]]) for t in range(NT)) for i in range(cap)
    )
    return groups, slot_lens


def _pack_core_inputs(x, w1, w2, counts, offs, groups, slot_lens):
    """Per-core DRAM blobs (bf16), shared xt per token group."""
    tloc = int(sum(slot_lens))
    soffs = np.concatenate([[0], np.cumsum(slot_lens)]).astype(int)

    xts = []
    for t in range(NT):
        xt = np.zeros((KC, 128, tloc), BF16NP)
        for i, e in enumerate(groups[t]):
            cnt = int(counts[e])
            if cnt == 0:
                continue
            seg = x[offs[e] : offs[e] + cnt].T.astype(BF16NP)  # [1024, cnt]
            xt[:, :, soffs[i] : soffs[i] + cnt] = seg.reshape(KC, 128, cnt)
        xts.append(xt)

    in_maps = []
    for c in range(N_CORES):
        t, q = divmod(c, NQ)
        w1b = np.empty((NSLOT, 128, NPAIR * 2 * KC * 128), BF16NP)
        w2b = np.empty((NSLOT, 128, MO * KI * 128), BF16NP)
        for i, e in enumerate(groups[t]):
            up = w1[e][:, q * IW : (q + 1) * IW]
            gate = w1[e][:, INTER + q * IW : INTER + (q + 1) * IW]
            hs = np.stack([up, gate], 0).astype(BF16NP)  # [2, 1024, 512]
            hs = hs.reshape(2, KC, 128, NPAIR, 128).transpose(2, 3, 0, 1, 4)
            w1b[i] = hs.reshape(128, NPAIR * 2 * KC * 128)
            sl = w2[e][q * IW : (q + 1) * IW, :].astype(BF16NP)  # [512, 1024]
            sl = sl.reshape(KI, 128, MO, 128).transpose(1, 2, 0, 3)
            w2b[i] = sl.reshape(128, MO * KI * 128)
        in_maps.append({"xt": xts[t], "w1b": w1b, "w2b": w2b})
    return in_maps


_LAST_RESULTS = {}  # exposed for test.py (exec time, trace paths)


def kernel(permuted_tokens, tokens_per_expert, w1, w2, _trace=False):
    from concourse.bass_utils import run_bass_kernel_spmd

    x = np.asarray(permuted_tokens, np.float32)
    counts = np.asarray(tokens_per_expert, np.int64)
    w1 = np.asarray(w1, np.float32)
    w2 = np.asarray(w2, np.float32)

    offs = np.zeros(GROUPS + 1, np.int64)
    offs[1:] = np.cumsum(counts)

    groups, slot_lens = _partition_experts(counts)
    nc = _get_program(slot_lens)
    in_maps = _pack_core_inputs(x, w1, w2, counts, offs, groups, slot_lens)

    kwargs = {}
    if _trace:
        kwargs = dict(trace=True, trace_cores=list(range(N_CORES)))
    res = run_bass_kernel_spmd(nc, in_maps, core_ids=list(range(N_CORES)), **kwargs)
    _LAST_RESULTS["res"] = res

    soffs = np.concatenate([[0], np.cumsum(slot_lens)]).astype(int)
    out = np.empty((NUM_TOKENS, HIDDEN), np.float32)
    for t in range(NT):
        acc = np.zeros((128, MO, int(sum(slot_lens))), np.float32)
        for q in range(NQ):
            acc += res.results[t * NQ + q]["yt"].astype(np.float32)
        ymat = acc.transpose(1, 0, 2).reshape(HIDDEN, -1)  # [1024, tloc]
        for i, e in enumerate(groups[t]):
            cnt = int(counts[e])
            if cnt == 0:
                continue
            out[offs[e] : offs[e] + cnt] = ymat[:, soffs[i] : soffs[i] + cnt].T
    return out


# revision 8
# speedup vs baseline: 1.8409x; 1.0225x over previous
"""Grouped MLP (MoE expert FFN) Bass kernel for 8 Trainium2 NeuronCores.

Problem: 4096 tokens sorted by expert (8 experts, uneven counts), per-expert
GLU MLP:  h = x @ w1[g]  (-> up|gate, 2*2048 cols);  a = silu(up)*gate;
y = a @ w2[g].

Sharding: 2-way token-parallel x 4-way tensor-parallel (INTER split), bf16.
Core (t, q) with t = c//4, q = c%4:
  - token group t owns 4 experts (balanced 4+4 partition of the counts),
  - inter slice q owns up/gate columns [q*512:(q+1)*512] of every expert
    and the matching w2 rows; its fc2 output is a partial sum of y.
Each core therefore runs an identical program over `slot` token segments
(common padded lengths across both groups); only the DRAM bytes differ.
The host converts everything to bf16, packs weight tiles in consumption
order, and sums the 4 partial y outputs per token group (host-side
reduction; no device collectives).

Device program per core (transposed feature-major space):
  fc1: for slot s, chunk c (<=512 tokens), pair p (128 inter cols):
       up/gate psum accumulated over 8 k-blocks; silu(up)*gate -> a (bf16)
  fc2: one slot behind fc1 (PE never waits on ACT/DVE):
       y^T psum over 4 k-blocks of a; copied bf16 to staging, DMA'd out.
"""

import sys

try:  # concourse normally comes from the container's PYTHONPATH
    import concourse  # noqa: F401
except ImportError:  # pragma: no cover - fallback for stripped env
    for _p in (
        "/root/.axon_site",
        "/root/.axon_site/_ro/trn_rl_repo",
        "/root/.axon_site/_ro/pypackages",
        "/opt/trn_rl_repo",
    ):
        if _p not in sys.path:
            sys.path.append(_p)

from contextlib import ExitStack

import numpy as np
import ml_dtypes

BF16NP = ml_dtypes.bfloat16

NUM_TOKENS = 4096
HIDDEN = 1024
INTER = 2048
GROUPS = 8
N_CORES = 8

NQ = 4              # inter-dim splits
NT = 2              # token-group splits
IW = INTER // NQ    # 512 inter cols per core
NPAIR = IW // 128   # 4 (up,gate) pair blocks
KC = HIDDEN // 128  # 8 k-blocks for fc1
KI = IW // 128      # 4 k-blocks for fc2
MO = HIDDEN // 128  # 8 output row-blocks of y
NSLOT = GROUPS // NT  # 4 expert slots per core


def _chunks(length):
    out, off = [], 0
    while off < length:
        c = min(512, length - off)
        out.append((off, c))
        off += c
    return out


_PROGRAM_CACHE: dict = {}


def _build_program(slot_lens):
    """Single-core Bass program; identical NEFF on all 8 cores."""
    import concourse.bass as bass  # noqa: F401
    import concourse.mybir as mybir
    import concourse.tile as tile
    from concourse import bacc

    f32 = mybir.dt.float32
    bf16 = mybir.dt.bfloat16
    silu = mybir.ActivationFunctionType.Silu

    tloc = sum(slot_lens)
    soffs = np.concatenate([[0], np.cumsum(slot_lens)]).astype(int)

    nc = bacc.Bacc("TRN2", target_bir_lowering=False, debug=False)

    xt_d = nc.dram_tensor("xt", [KC, 128, tloc], bf16, kind="ExternalInput").ap()
    w1_d = nc.dram_tensor(
        "w1b", [NSLOT, 128, NPAIR * 2 * KC * 128], bf16, kind="ExternalInput"
    ).ap()
    w2_d = nc.dram_tensor(
        "w2b", [NSLOT, 128, MO * KI * 128], bf16, kind="ExternalInput"
    ).ap()
    yt_d = nc.dram_tensor("yt", [128, MO, tloc], bf16, kind="ExternalOutput").ap()

    with tile.TileContext(nc) as tc, ExitStack() as ctx:
        xp = ctx.enter_context(tc.tile_pool(name="x", bufs=1))
        w1p = ctx.enter_context(tc.tile_pool(name="w1", bufs=2))
        w2p = ctx.enter_context(tc.tile_pool(name="w2", bufs=2))
        ap_ = ctx.enter_context(tc.tile_pool(name="a", bufs=2))
        yp = ctx.enter_context(tc.tile_pool(name="y", bufs=2))
        tp = ctx.enter_context(tc.tile_pool(name="tmp", bufs=4))
        pup = ctx.enter_context(tc.tile_pool(name="pu", bufs=2, space="PSUM"))
        pgp = ctx.enter_context(tc.tile_pool(name="pg", bufs=2, space="PSUM"))
        pyp = ctx.enter_context(tc.tile_pool(name="py", bufs=3, space="PSUM"))

        x_sb = xp.tile([128, KC, tloc], bf16)

        def emit_fc2(s, llen, w2t, a_t, copy_flip):
            y_t = yp.tile([128, MO, llen], bf16)
            for mo in range(MO):
                for coff, clen in _chunks(llen):
                    py = pyp.tile([128, clen], f32)
                    for ki in range(KI):
                        nc.tensor.matmul(
                            py[:, :],
                            w2t[:, (mo * KI + ki) * 128 : (mo * KI + ki + 1) * 128],
                            a_t[:, ki, coff : coff + clen],
                            start=(ki == 0),
                            stop=(ki == KI - 1),
                        )
                    dst = y_t[:, mo, coff : coff + clen]
                    if copy_flip[0]:
                        nc.scalar.copy(dst, py[:, :])
                    else:
                        nc.vector.tensor_copy(dst, py[:, :])
                    copy_flip[0] = not copy_flip[0]
                if mo % 2 == 1:  # stream out as mo-pairs complete
                    nc.sync.dma_start(
                        out=yt_d[:, mo - 1 : mo + 1, soffs[s] : soffs[s] + llen],
                        in_=y_t[:, mo - 1 : mo + 1, :],
                    )

        pending = None
        copy_flip = [True]
        for s in range(NSLOT):
            llen = int(slot_lens[s])
            if llen == 0:
                continue
            w1t = w1p.tile([128, NPAIR * 2 * KC * 128], bf16)
            hw = KC * 128  # one (pair, half) group of 8 weight tiles
            # first half-pair's weights + this slot's x land first so PE
            # starts early; remaining weight groups trail behind the x stream
            nc.sync.dma_start(out=w1t[:, 0:hw], in_=w1_d[s, :, 0:hw])
            for kc in range(KC):
                nc.sync.dma_start(
                    out=x_sb[:, kc, soffs[s] : soffs[s] + llen],
                    in_=xt_d[kc, :, soffs[s] : soffs[s] + llen],
                )
            for g in range(1, 2 * NPAIR):
                nc.sync.dma_start(
                    out=w1t[:, g * hw : (g + 1) * hw],
                    in_=w1_d[s, :, g * hw : (g + 1) * hw],
                )
            w2t = w2p.tile([128, MO * KI * 128], bf16)
            nc.sync.dma_start(out=w2t[:, :], in_=w2_d[s])

            a_t = ap_.tile([128, KI, llen], bf16)
            chunks = _chunks(llen)
            for p in range(NPAIR):
                pus = [pup.tile([128, clen], f32, name="pu") for _, clen in chunks]
                pgs = [pgp.tile([128, clen], f32, name="pg") for _, clen in chunks]
                for h, pss in ((0, pus), (1, pgs)):
                    tbase = (p * 2 + h) * KC
                    for kc in range(KC):
                        for ci, (coff, clen) in enumerate(chunks):
                            nc.tensor.matmul(
                                pss[ci][:, :],
                                w1t[:, (tbase + kc) * 128 : (tbase + kc + 1) * 128],
                                x_sb[:, kc, soffs[s] + coff : soffs[s] + coff + clen],
                                start=(kc == 0),
                                stop=(kc == KC - 1),
                            )
                for ci, (coff, clen) in enumerate(chunks):
                    tmp = tp.tile([128, clen], f32)
                    nc.scalar.activation(tmp[:, :], pus[ci][:, :], silu)
                    nc.vector.tensor_mul(
                        a_t[:, p, coff : coff + clen], tmp[:, :], pgs[ci][:, :]
                    )

            if pending is not None:
                emit_fc2(*pending, copy_flip)
            pending = (s, llen, w2t, a_t)
        if pending is not None:
            emit_fc2(*pending, copy_flip)

    nc.compile()
    return nc


def _get_program(slot_lens):
    key = tuple(int(v) for v in slot_lens)
    if key not in _PROGRAM_CACHE:
        _PROGRAM_CACHE[key] = _build_program(key)
    return _PROGRAM_CACHE[key]


def _partition_experts(counts):
    """Split experts into NT groups of GROUPS//NT, balancing token sums."""
    order = np.argsort(-counts, kind="stable")
    groups = [[] for _ in range(NT)]
    sums = [0] * NT
    cap = GROUPS // NT
    for e in order:
        cand = sorted(range(NT), key=lambda t: (sums[t],))
        for t in cand:
            if len(groups[t]) < cap:
                groups[t].append(int(e))
                sums[t] += int(counts[e])
                break
    # within each group, biggest expert first -> slot i
    for t in range(NT):
        groups[t].sort(key=lambda e: -int(counts[e]))
    slot_lens = tuple(
        max(int(counts[groups[t][i]]) for t in range(NT)) for i in range(cap)
    )
    return groups, slot_lens


def _pack_core_inputs(x, w1, w2, counts, offs, groups, slot_lens):
    """Per-core DRAM blobs (bf16), shared xt per token group."""
    tloc = int(sum(slot_lens))
    soffs = np.concatenate([[0], np.cumsum(slot_lens)]).astype(int)

    xts = []
    for t in range(NT):
        xt = np.zeros((KC, 128, tloc), BF16NP)
        for i, e in enumerate(groups[t]):
            cnt = int(counts[e])
            if cnt == 0:
                continue
            seg = x[offs[e] : offs[e] + cnt].T.astype(BF16NP)  # [1024, cnt]
            xt[:, :, soffs[i] : soffs[i] + cnt] = seg.reshape(KC, 128, cnt)
        xts.append(xt)

    in_maps = []
    for c in range(N_CORES):
        t, q = divmod(c, NQ)
        w1b = np.empty((NSLOT, 128, NPAIR * 2 * KC * 128), BF16NP)
        w2b = np.empty((NSLOT, 128, MO * KI * 128), BF16NP)
        for i, e in enumerate(groups[t]):
            up = w1[e][:, q * IW : (q + 1) * IW]
            gate = w1[e][:, INTER + q * IW : INTER + (q + 1) * IW]
            hs = np.stack([up, gate], 0).astype(BF16NP)  # [2, 1024, 512]
            hs = hs.reshape(2, KC, 128, NPAIR, 128).transpose(2, 3, 0, 1, 4)
            w1b[i] = hs.reshape(128, NPAIR * 2 * KC * 128)
            sl = w2[e][q * IW : (q + 1) * IW, :].astype(BF16NP)  # [512, 1024]
            sl = sl.reshape(KI, 128, MO, 128).transpose(1, 2, 0, 3)
            w2b[i] = sl.reshape(128, MO * KI * 128)
        in_maps.append({"xt": xts[t], "w1b": w1b, "w2b": w2b})
    return in_maps


_LAST_RESULTS = {}  # exposed for test.py (exec time, trace paths)


def kernel(permuted_tokens, tokens_per_expert, w1, w2, _trace=False):
    from concourse.bass_utils import run_bass_kernel_spmd

    x = np.asarray(permuted_tokens, np.float32)
    counts = np.asarray(tokens_per_expert, np.int64)
    w1 = np.asarray(w1, np.float32)
    w2 = np.asarray(w2, np.float32)

    offs = np.zeros(GROUPS + 1, np.int64)
    offs[1:] = np.cumsum(counts)

    groups, slot_lens = _partition_experts(counts)
    nc = _get_program(slot_lens)
    in_maps = _pack_core_inputs(x, w1, w2, counts, offs, groups, slot_lens)

    kwargs = {}
    if _trace:
        kwargs = dict(trace=True, trace_cores=list(range(N_CORES)))
    res = run_bass_kernel_spmd(nc, in_maps, core_ids=list(range(N_CORES)), **kwargs)
    _LAST_RESULTS["res"] = res

    soffs = np.concatenate([[0], np.cumsum(slot_lens)]).astype(int)
    out = np.empty((NUM_TOKENS, HIDDEN), np.float32)
    for t in range(NT):
        acc = np.zeros((128, MO, int(sum(slot_lens))), np.float32)
        for q in range(NQ):
            acc += res.results[t * NQ + q]["yt"].astype(np.float32)
        ymat = acc.transpose(1, 0, 2).reshape(HIDDEN, -1)  # [1024, tloc]
        for i, e in enumerate(groups[t]):
            cnt = int(counts[e])
            if cnt == 0:
                continue
            out[offs[e] : offs[e] + cnt] = ymat[:, soffs[i] : soffs[i] + cnt].T
    return out
